# revision 4
# baseline (speedup 1.0000x reference)
"""Trainium2 Bass kernel for BaselineDNN (ragged embedding pooling + MLP).

Data-parallel over batch (8 cores). Per core 512 rows, 4 blocks of 128.

Host prep:
  - rows sorted by length, stratified-sharded (core c takes sorted rank c::8)
    so every core sees the same length distribution and the SPMD program
    (shared gather schedule) wastes little on padding.
  - the fp32 [50000, 300] table is repacked to fp16 [50004, 384] (768B rows,
    256B-multiple for dma_gather):  [pad_neg, pad_zero, emb..., pad_zero,
    pad_neg].  dma_gather indices are int16 (<32768) so gathers read one of
    two overlapping windows: lo = rows [0, 32768), hi = rows [17236, 50004).
    Tokens in the overlap are assigned to balance per-row lo/hi counts.
  - per (block, window) the host builds "waves": wave w = one token per row
    (one SBUF partition each), padded with a pad row.  Block 0 (shortest
    rows) pads with -1.0 and its avg-pool is corrected exactly on device;
    blocks 1..3 pad with 0.0.
  - per-block int16 index arrays are loaded as separate DMAs so the first
    gather only waits for its own block's indices.

Device:
  - chained dma_gather (fp16, W<=8 waves = up to 1024 tokens each) spread
    over 4 SWDGE queues.  The Q7 descriptor generation (~4.8ns/desc) is the
    critical path; everything else overlaps under it.
  - DVE: per-gather fp16 max accumulate (unit-stride tensor_tensor into
    [128, W, 384] accumulator), one strided reduce_max per block.
  - PE: per-wave identity matmuls accumulate the sum pool in f32 PSUM
    (removes the sum work from DVE); then fp16 matmuls for the MLP.
  - ACT: avg = (psum + corr) * (1/len) read directly from PSUM; relu; logits.
  - per-block output DMA so the kernel tail is just the last (shortest) block.
"""
import sys

sys.path.insert(0, "/opt/trn_rl_repo")

import numpy as np

import concourse.bacc as bacc
import concourse.bass as bass
import concourse.mybir as mybir
import concourse.tile as tile
from concourse.bass_utils import run_bass_kernel_spmd
from concourse.masks import make_identity

VOCAB, EMB_DIM, HIDDEN, NUM_CLASSES = 50000, 300, 1000, 5
B, MAX_LEN = 4096, 128
NCORES = 8
ROWS_PER_CORE = B // NCORES          # 512
NBLOCKS = ROWS_PER_CORE // 128       # 4
E_PAD = 384                          # fp16 row: 768B (256B multiple)
DEV_ROWS = VOCAB + 4                 # [pad_neg, pad_zero, emb..., pad_zero, pad_neg]
LO_SIZE = 32768
HI_BASE = DEV_ROWS - 32768           # 17236
PAD_NEG = -1.0
W_MAX = 8                            # waves per dma_gather
NQ = 4                               # SWDGE queues
KC, MC = 120, 125                    # matmul k-chunk (600=5*120) / m-chunk (1000=8*125)
BLOCK_ORDER = (3, 2, 1, 0)           # longest first (pipeline fill), shortest last (tail)

_dt = mybir.dt


def _plan(x, lengths):
    x = np.asarray(x)
    lengths = np.asarray(lengths).astype(np.int64)
    order = np.argsort(lengths, kind="stable")
    core_rows = [order[c::NCORES] for c in range(NCORES)]

    # per core/row: balanced lo/hi token lists (local window indices)
    lo_toks = [[None] * ROWS_PER_CORE for _ in range(NCORES)]
    hi_toks = [[None] * ROWS_PER_CORE for _ in range(NCORES)]
    for c in range(NCORES):
        for r, g in enumerate(core_rows[c]):
            d = x[g, : lengths[g]].astype(np.int64) + 2  # device row id
            forced_lo = d[d < HI_BASE]
            forced_hi = d[d >= LO_SIZE]
            flex = d[(d >= HI_BASE) & (d < LO_SIZE)]
            t = len(d)
            lo_take = int(np.clip((t + 1) // 2 - len(forced_lo), 0, len(flex)))
            lo = np.concatenate([forced_lo, flex[:lo_take]])
            hi = np.concatenate([forced_hi, flex[lo_take:]])
            lo_toks[c][r] = lo.astype(np.int16)
            hi_toks[c][r] = (hi - HI_BASE).astype(np.int16)

    C_lo, C_hi = [], []
    for b in range(NBLOCKS):
        rs = range(b * 128, (b + 1) * 128)
        C_lo.append(max(len(lo_toks[c][r]) for c in range(NCORES) for r in rs))
        C_hi.append(max(len(hi_toks[c][r]) for c in range(NCORES) for r in rs))

    # per-block gather schedule, shared across cores: (n_waves, col_off, is_lo)
    # col_off is within the block's own idx array.
    sched = {b: [] for b in range(NBLOCKS)}
    block_cols = {}
    for b in range(NBLOCKS):
        col_off = 0
        for is_lo in (True, False):
            C = C_lo[b] if is_lo else C_hi[b]
            w0 = 0
            while w0 < C:
                w = min(W_MAX, C - w0)
                sched[b].append((w, col_off, is_lo))
                col_off += w * 8
                w0 += w
        # gathers sorted so the first has the block-max wave count (acc init)
        sched[b].sort(key=lambda t: -t[0])
        block_cols[b] = col_off

    def pad_idx(b, is_lo):
        if is_lo:
            return 0 if b == 0 else 1
        return 32767 if b == 0 else 32766

    idx_arrs = {b: np.zeros((NCORES, 128, block_cols[b]), np.int16)
                for b in range(NBLOCKS)}
    scale = np.zeros((NCORES, 128, NBLOCKS), np.float32)
    bias = np.zeros((NCORES, 128, NBLOCKS), np.float32)
    for c in range(NCORES):
        for b in range(NBLOCKS):
            wl = np.full((C_lo[b], 128), pad_idx(b, True), np.int16)
            wh = np.full((C_hi[b], 128), pad_idx(b, False), np.int16)
            for p in range(128):
                r = b * 128 + p
                lo, hi = lo_toks[c][r], hi_toks[c][r]
                wl[: len(lo), p] = lo
                wh[: len(hi), p] = hi
            ln = lengths[core_rows[c][b * 128 : (b + 1) * 128]].astype(np.float32)
            scale[c, :, b] = 1.0 / ln
            if b == 0:
                npad = (C_lo[b] + C_hi[b]) - ln  # each pad contributed PAD_NEG
                bias[c, :, b] = -PAD_NEG * npad / ln
            cur = {0: 0, 1: 0}
            for (w, off, is_lo) in sched[b]:
                mat = wl if is_lo else wh
                w0 = cur[0 if is_lo else 1]
                cur[0 if is_lo else 1] = w0 + w
                flat = mat[w0 : w0 + w].reshape(-1)
                wrapped = flat.reshape(-1, 16).T
                idx_arrs[b][c, :, off : off + w * 8] = np.tile(wrapped, (8, 1))

    inv_perm = np.empty(B, np.int64)
    inv_perm[np.concatenate(core_rows)] = np.arange(B)
    return dict(sched=sched, block_cols=block_cols, idx=idx_arrs,
                scale=scale, bias=bias, inv_perm=inv_perm)


def _build_nc(sched, block_cols):
    nc = bacc.Bacc("TRN2", target_bir_lowering=False, debug=False,
                   num_swdge_queues=NQ)
    table = nc.declare_dram_parameter("table", [DEV_ROWS, E_PAD], _dt.float16, isOutput=False)
    idx_d = {b: nc.declare_dram_parameter(f"idx{b}", [128, block_cols[b]], _dt.int16,
                                          isOutput=False) for b in range(NBLOCKS)}
    sb = nc.declare_dram_parameter("sb", [128, 2 * NBLOCKS], _dt.float32, isOutput=False)
    w1 = nc.declare_dram_parameter("w1", [2 * EMB_DIM, HIDDEN], _dt.float16, isOutput=False)
    b1 = nc.declare_dram_parameter("b1", [HIDDEN], _dt.float32, isOutput=False)
    w2 = nc.declare_dram_parameter("w2", [HIDDEN, NUM_CLASSES], _dt.float16, isOutput=False)
    b2 = nc.declare_dram_parameter("b2", [NUM_CLASSES], _dt.float32, isOutput=False)
    out = nc.declare_dram_parameter("out", [ROWS_PER_CORE, NUM_CLASSES], _dt.float32, isOutput=True)

    table_lo = table[0:LO_SIZE, :]
    table_hi = table[HI_BASE:DEV_ROWS, :]

    qctr = [0]

    def next_q():
        q = qctr[0] % NQ
        qctr[0] += 1
        return q

    with tile.TileContext(nc) as tc:
        with (
            tc.tile_pool(name="const", bufs=1) as cpool,
            tc.tile_pool(name="gather", bufs=8) as gpool,
            tc.tile_pool(name="acc", bufs=2) as apool,
            tc.tile_pool(name="red", bufs=2) as rpool,
            tc.tile_pool(name="mlp", bufs=2) as mpool,
            tc.tile_pool(name="psum", bufs=2, space="PSUM") as ppool,
            tc.tile_pool(name="psum2", bufs=2, space="PSUM") as ppool2,
            tc.tile_pool(name="psums", bufs=2, space="PSUM") as ppool3,
        ):
            # per-block idx loads, in gather order, before everything else
            idx_t = {}
            for b in BLOCK_ORDER:
                t = cpool.tile([128, block_cols[b]], _dt.int16, tag=f"idx{b}")
                nc.sync.dma_start(out=t[:], in_=idx_d[b][:])
                idx_t[b] = t
            sb_t = cpool.tile([128, 2 * NBLOCKS], _dt.float32)
            nc.sync.dma_start(out=sb_t[:], in_=sb[:])
            w1_t = cpool.tile([KC, 5 * HIDDEN], _dt.float16)
            for k in range(5):
                nc.sync.dma_start(out=w1_t[:, k * HIDDEN : (k + 1) * HIDDEN],
                                  in_=w1[k * KC : (k + 1) * KC, :])
            b1_t = cpool.tile([MC, 8], _dt.float32)
            nc.sync.dma_start(out=b1_t[:], in_=b1[:].rearrange("(m p) -> p m", p=MC))
            w2_t = cpool.tile([MC, 8 * NUM_CLASSES], _dt.float16)
            for m in range(8):
                nc.sync.dma_start(out=w2_t[:, m * NUM_CLASSES : (m + 1) * NUM_CLASSES],
                                  in_=w2[m * MC : (m + 1) * MC, :])
            b2_t = cpool.tile([NUM_CLASSES, 1], _dt.float32)
            nc.sync.dma_start(out=b2_t[:], in_=b2[:, None])
            ident = cpool.tile([128, 128], _dt.float16)
            make_identity(nc, ident[:])

            for b in BLOCK_ORDER:
                gathers = sched[b]
                wa = gathers[0][0]   # block-max wave count (sorted desc)
                max_acc = apool.tile([128, W_MAX, E_PAD], _dt.float16, tag="macc")
                psum_sum = ppool3.tile([128, EMB_DIM], _dt.float32, tag="ps",
                                       space="PSUM")
                n_g = len(gathers)

                for gi, (w, off, is_lo) in enumerate(gathers):
                    g_t = gpool.tile([128, W_MAX, E_PAD], _dt.float16, tag="g")
                    src = table_lo if is_lo else table_hi
                    nc.gpsimd.dma_gather(
                        g_t[:, :w, :], src, idx_t[b][:, off : off + w * 8],
                        w * 128, w * 128, E_PAD, single_packet=False,
                        queue_num=next_q(),
                    )
                    # DVE: max accumulate (full tile, unit stride)
                    if gi == 0:
                        nc.vector.tensor_copy(out=max_acc[:, :w, :], in_=g_t[:, :w, :])
                    else:
                        nc.vector.tensor_tensor(
                            out=max_acc[:, :w, 0:EMB_DIM],
                            in0=max_acc[:, :w, 0:EMB_DIM],
                            in1=g_t[:, :w, 0:EMB_DIM],
                            op=mybir.AluOpType.max)
                    # PE: sum accumulate, one identity matmul per wave
                    wave0 = gi == 0
                    for wv in range(w):
                        nc.tensor.matmul(
                            psum_sum[:],
                            ident[:],
                            g_t[:, wv, 0:EMB_DIM],
                            start=(wave0 and wv == 0),
                            stop=(gi == n_g - 1 and wv == w - 1),
                        )

                rep = rpool.tile([128, 2 * EMB_DIM], _dt.float16, tag="rep")
                nc.vector.reduce_max(
                    rep[:, EMB_DIM : 2 * EMB_DIM],
                    max_acc[:, :wa, 0:EMB_DIM].rearrange("p w e -> p e w"),
                    axis=mybir.AxisListType.X,
                )
                nc.scalar.activation(
                    rep[:, 0:EMB_DIM], psum_sum[:],
                    mybir.ActivationFunctionType.Identity,
                    bias=sb_t[:, NBLOCKS + b : NBLOCKS + b + 1],
                    scale=sb_t[:, b : b + 1],
                )

                # repT [600, 128] as 5 chunks of [120, 128]
                repT = mpool.tile([KC, 5 * 128], _dt.float16, tag="repT")
                for k in range(5):
                    tp = ppool.tile([KC, 128], _dt.float16, tag="tp", space="PSUM")
                    nc.tensor.transpose(out=tp[:], in_=rep[:, k * KC : (k + 1) * KC],
                                        identity=ident[:])
                    nc.vector.tensor_copy(out=repT[:, k * 128 : (k + 1) * 128], in_=tp[:])

                hT = mpool.tile([MC, 8 * 128], _dt.float16, tag="hT")
                for m in range(8):
                    hp = ppool.tile([MC, 128], _dt.float32, tag="hp", space="PSUM")
                    for k in range(5):
                        nc.tensor.matmul(
                            hp[:],
                            w1_t[:, k * HIDDEN + m * MC : k * HIDDEN + (m + 1) * MC],
                            repT[:, k * 128 : (k + 1) * 128],
                            start=(k == 0), stop=(k == 4),
                        )
                    nc.scalar.activation(
                        hT[:, m * 128 : (m + 1) * 128], hp[:],
                        mybir.ActivationFunctionType.Relu,
                        bias=b1_t[:, m : m + 1],
                    )

                lp = ppool2.tile([NUM_CLASSES, 128], _dt.float32, tag="lp", space="PSUM")
                for m in range(8):
                    nc.tensor.matmul(
                        lp[:],
                        w2_t[:, m * NUM_CLASSES : (m + 1) * NUM_CLASSES],
                        hT[:, m * 128 : (m + 1) * 128],
                        start=(m == 0), stop=(m == 7),
                    )
                logitsT = rpool.tile([NUM_CLASSES, 128], _dt.float32, tag="lg")
                nc.scalar.activation(
                    logitsT[:], lp[:],
                    mybir.ActivationFunctionType.Identity,
                    bias=b2_t[:, 0:1],
                )
                nc.sync.dma_start(
                    out=out[b * 128 : (b + 1) * 128, :].rearrange("r c -> c r"),
                    in_=logitsT[:])
    nc.compile()
    return nc


def kernel(x, lengths, emb_table, W1, b1, W2, b2, _trace=False, _trace_cores=None):
    x = np.asarray(x)
    lengths = np.asarray(lengths)
    plan = _plan(x, lengths)
    nc = _build_nc(plan["sched"], plan["block_cols"])

    table_dev = np.zeros((DEV_ROWS, E_PAD), np.float16)
    table_dev[0, :] = PAD_NEG
    table_dev[-1, :] = PAD_NEG
    table_dev[2 : VOCAB + 2, :EMB_DIM] = np.asarray(emb_table, np.float32).astype(np.float16)

    in_maps = []
    for c in range(NCORES):
        sbv = np.concatenate([plan["scale"][c], plan["bias"][c]], axis=1).astype(np.float32)
        im = {
            "table": table_dev,
            "sb": sbv,
            "w1": np.asarray(W1, np.float32).astype(np.float16),
            "b1": np.asarray(b1, np.float32),
            "w2": np.asarray(W2, np.float32).astype(np.float16),
            "b2": np.asarray(b2, np.float32),
        }
        for b in range(NBLOCKS):
            im[f"idx{b}"] = np.ascontiguousarray(plan["idx"][b][c])
        in_maps.append(im)
    kw = {}
    if _trace:
        kw = dict(trace=True, trace_cores=_trace_cores or [0])
    res = run_bass_kernel_spmd(nc, in_maps, core_ids=list(range(NCORES)), **kw)
    logits_sorted = np.concatenate([res.results[c]["out"] for c in range(NCORES)], axis=0)
    logits = logits_sorted[plan["inv_perm"]]
    if _trace:
        return logits, res
    return logits
